# revision 5
# baseline (speedup 1.0000x reference)
"""Trainium2 Bass kernel for ConditionalStructureEncoder (2-layer GCN + VAE heads).

Strategy (8 NeuronCores, SPMD):
  - Destination-node sharding: core c owns dst nodes [c*6250, (c+1)*6250).
  - Host precomputes: homophily MLP folded into node features (H0 = x + hom),
    GCN symmetric norm (deg^-1/2), and per-core edge lists sorted by
    (dst block, src-half), padded to 128-edge chunks with uniform static
    chunk counts across cores (SPMD requires an identical program).
  - Aggregation A_hat @ H is computed as PE matmuls: for each 128-edge chunk,
    gathered source rows G [e,feat] (via SWDGE dma_gather, batched per range
    of 4 dst blocks) are multiplied with a one-hot selection matrix
    S[e,d] = (iota==localdst)*norm built in one DVE tensor_scalar op.
    Self-loops are one extra "diagonal" chunk per block read sequentially.
  - The linear layer W is applied AFTER aggregation (linearity), so the
    gather table holds raw activations and per-block matmuls are tiny.
  - Inter-layer halo exchange: AllGather of the layer-1 activations in bf16.
  - mu/logvar heads computed per block in transposed layout; host departs.

Self-contained: hardcodes all shapes from the problem spec.
"""
import os
import sys

sys.path.insert(0, "/opt/trn_rl_repo")

import numpy as np

import concourse.bass as bass
import concourse.mybir as mybir
from concourse import bacc, tile
from concourse.bass_utils import run_bass_kernel_spmd
from concourse.masks import make_identity

P = 128
N_NODES = 50000
F = 128
L = 64
CORES = 8
SHARD = N_NODES // CORES            # 6250
NBLK = (SHARD + P - 1) // P         # 49 blocks (48 full + 1 of 106)
LAST_BLK_ROWS = SHARD - (NBLK - 1) * P  # 106
RPB = 4                             # dst blocks per gather range
NRANGES = (NBLK + RPB - 1) // RPB   # 13
SPLIT = 32768                       # int16 gather index limit
PAD_SHARD = NBLK * P                # 6272

_cache = {}

# info for test harness
last_run_info = {}


def _install_trace_shims():
    """Register the NTFF profile hook (missing antenv.axon_hooks in this
    container) and neuter the artifact upload so trace=True works."""
    import types
    import contextlib
    import ctypes

    import antenv
    from concourse import bass_utils as bu

    if "antenv.axon_hooks" not in sys.modules:
        mod = types.ModuleType("antenv.axon_hooks")
        _state = {"hook": None}

        def set_axon_ntff_profile_hook(h):
            _state["hook"] = h

        def get_axon_ntff_profile_hook():
            return _state["hook"]

        mod.set_axon_ntff_profile_hook = set_axon_ntff_profile_hook
        mod.get_axon_ntff_profile_hook = get_axon_ntff_profile_hook
        sys.modules["antenv.axon_hooks"] = mod
        antenv.axon_hooks = mod

        lib = ctypes.CDLL("/opt/axon/libaxon_pjrt.so")
        lib.axon_start_nrt_profile.argtypes = [
            ctypes.POINTER(ctypes.c_int64), ctypes.c_size_t]
        lib.axon_start_nrt_profile.restype = ctypes.c_int64
        lib.axon_stop_nrt_profile.argtypes = [ctypes.c_char_p]
        lib.axon_stop_nrt_profile.restype = ctypes.c_int64

        @contextlib.contextmanager
        def _hook(output_dir, device_ids):
            import jax
            jax.devices()
            if device_ids:
                ids = (ctypes.c_int64 * len(device_ids))(*device_ids)
                rc = lib.axon_start_nrt_profile(ids, len(device_ids))
            else:
                rc = lib.axon_start_nrt_profile(None, 0)
            if rc != 0:
                raise RuntimeError(f"axon_start_nrt_profile rc={rc}")
            try:
                yield
            finally:
                n = lib.axon_stop_nrt_profile(str(output_dir).encode())
                print(f"profile: {n} file(s) written to {output_dir}")

        set_axon_ntff_profile_hook(_hook)

    bu.upload_artifacts = lambda tmpdir: tmpdir


def _host_prepare(x, edge_index, homophily_cond, hw1, hb1, hw2, hb2):
    """Compute H0 = x + hom and all per-core edge structures."""
    # homophily embedding (tiny MLP) on host, float64 for accuracy
    hc = homophily_cond.astype(np.float64)[None, :]
    hom = np.maximum(hc @ hw1.astype(np.float64) + hb1, 0.0) @ hw2.astype(np.float64) + hb2
    H0 = (x.astype(np.float64) + hom).astype(np.float32)  # [N, F]

    src = edge_index[0].astype(np.int64)
    dst = edge_index[1].astype(np.int64)
    E = src.shape[0]

    # GCN norm with self-loops: deg = in-degree over dst (+1 self-loop)
    deg = np.bincount(dst, minlength=N_NODES).astype(np.float64) + 1.0
    dis = 1.0 / np.sqrt(deg)
    norm_e = (dis[src] * dis[dst]).astype(np.float32)
    dis2 = (dis * dis).astype(np.float32)

    core = dst // SHARD
    localdst = dst % SHARD
    blk = localdst // P
    reldst = (localdst % P).astype(np.float32)
    isB = (src >= SPLIT).astype(np.int64)
    group = (core * NBLK + blk) * 2 + isB
    order = np.argsort(group, kind="stable")
    g_sorted = group[order]
    src_sorted = src[order]
    rel_sorted = reldst[order]
    nrm_sorted = norm_e[order]
    counts = np.bincount(group, minlength=CORES * NBLK * 2)
    starts = np.zeros(CORES * NBLK * 2 + 1, np.int64)
    np.cumsum(counts, out=starts[1:])

    cnt = counts.reshape(CORES, NBLK, 2)
    nch = (cnt + P - 1) // P                       # chunks per (core, blk, half)
    nch_s = nch.max(axis=0)                        # static chunk counts [NBLK, 2]

    # per-range static layout
    rng_blocks = [list(range(r * RPB, min((r + 1) * RPB, NBLK))) for r in range(NRANGES)]
    nchA_rng = [int(sum(nch_s[b, 0] for b in blks)) for blks in rng_blocks]
    nchB_rng = [int(sum(nch_s[b, 1] for b in blks)) for blks in rng_blocks]

    per_core = []
    for c in range(CORES):
        idxA_cols, idxB_cols = [], []
        ld_cols, nm_cols = [], []
        for r, blks in enumerate(rng_blocks):
            ldA_blk, nmA_blk, ldB_blk, nmB_blk = [], [], [], []
            for b in blks:
                for half, (idx_cols, ld_b, nm_b) in enumerate(
                    ((idxA_cols, ldA_blk, nmA_blk), (idxB_cols, ldB_blk, nmB_blk))
                ):
                    g = (c * NBLK + b) * 2 + half
                    s, e = starts[g], starts[g + 1]
                    n_pad = int(nch_s[b, half]) * P
                    sidx = np.zeros(n_pad, np.int64)
                    sidx[: e - s] = src_sorted[s:e] - (SPLIT if half else 0)
                    ld = np.full(n_pad, -1.0, np.float32)
                    ld[: e - s] = rel_sorted[s:e]
                    nm = np.zeros(n_pad, np.float32)
                    nm[: e - s] = nrm_sorted[s:e]
                    idx_cols.append(sidx.astype(np.int16))
                    # [P, nch] column-per-chunk layout (edge j -> [j%128, j//128])
                    ld_b.append(ld.reshape(-1, P).T)
                    nm_b.append(nm.reshape(-1, P).T)
            ld_cols.append(np.concatenate(ldA_blk + ldB_blk, axis=1))
            nm_cols.append(np.concatenate(nmA_blk + nmB_blk, axis=1))

        def wrap_idx(cols):
            flat = np.concatenate(cols) if cols else np.zeros(0, np.int16)
            # j -> [j%16, j//16], replicated to 128 partitions
            m = flat.reshape(-1, 16).T
            return np.tile(m, (8, 1))

        # build per-range wrapped index blocks, concatenated on free dim
        idxA_wrapped, idxB_wrapped = [], []
        ia = ib = 0
        for r, blks in enumerate(rng_blocks):
            na = sum(int(nch_s[b, 0]) for b in blks)
            nb = sum(int(nch_s[b, 1]) for b in blks)
            idxA_wrapped.append(wrap_idx(idxA_cols[ia: ia + len(blks)]))
            idxB_wrapped.append(wrap_idx(idxB_cols[ib: ib + len(blks)]))
            ia += len(blks)
            ib += len(blks)
        idxA = np.concatenate(idxA_wrapped, axis=1)
        idxB = np.concatenate(idxB_wrapped, axis=1)
        ldm = np.concatenate(ld_cols, axis=1)
        nmm = np.concatenate(nm_cols, axis=1)

        selfrows = np.zeros((PAD_SHARD, F), np.float32)
        selfrows[:SHARD] = H0[c * SHARD: (c + 1) * SHARD]
        d2 = np.zeros((PAD_SHARD, 1), np.float32)
        d2[:SHARD, 0] = dis2[c * SHARD: (c + 1) * SHARD]

        per_core.append(dict(idxA=idxA, idxB=idxB, ldm=ldm, nmm=nmm,
                             selfrows=selfrows, dis2own=d2))

    meta = dict(nch_s=nch_s, rng_blocks=rng_blocks,
                nchA_rng=nchA_rng, nchB_rng=nchB_rng)
    return H0, per_core, meta


def _build_program(meta, phase="full"):
    nch_s = meta["nch_s"]
    rng_blocks = meta["rng_blocks"]
    nchA_rng = meta["nchA_rng"]
    nchB_rng = meta["nchB_rng"]
    SA_tot = sum(n * P // 16 for n in nchA_rng)
    SB_tot = sum(n * P // 16 for n in nchB_rng)
    NCH_tot = sum(nchA_rng) + sum(nchB_rng)
    f32 = mybir.dt.float32
    bf16 = mybir.dt.bfloat16

    nc = bacc.Bacc("TRN2", target_bir_lowering=False, debug=False,
                   num_devices=CORES)

    table1 = nc.dram_tensor("table1", [N_NODES, F], f32, kind="ExternalInput")
    idxA_d = nc.dram_tensor("idxA", [P, SA_tot], mybir.dt.int16, kind="ExternalInput")
    idxB_d = nc.dram_tensor("idxB", [P, SB_tot], mybir.dt.int16, kind="ExternalInput")
    ld_d = nc.dram_tensor("ldm", [P, NCH_tot], f32, kind="ExternalInput")
    nm_d = nc.dram_tensor("nmm", [P, NCH_tot], f32, kind="ExternalInput")
    selfrows_d = nc.dram_tensor("selfrows", [PAD_SHARD, F], f32, kind="ExternalInput")
    dis2_d = nc.dram_tensor("dis2own", [PAD_SHARD, 1], f32, kind="ExternalInput")
    gw1_d = nc.dram_tensor("gw1", [F, F], f32, kind="ExternalInput")
    gw2_d = nc.dram_tensor("gw2", [F, F], f32, kind="ExternalInput")
    gb1_d = nc.dram_tensor("gb1", [F, 1], f32, kind="ExternalInput")
    gb2_d = nc.dram_tensor("gb2", [F, 1], f32, kind="ExternalInput")
    muw_d = nc.dram_tensor("muw", [F, L], f32, kind="ExternalInput")
    lvw_d = nc.dram_tensor("lvw", [F, L], f32, kind="ExternalInput")
    mub_d = nc.dram_tensor("mub", [L, 1], f32, kind="ExternalInput")
    lvb_d = nc.dram_tensor("lvb", [L, 1], f32, kind="ExternalInput")
    muT_o = nc.dram_tensor("muT", [L, SHARD], f32, kind="ExternalOutput")
    lvT_o = nc.dram_tensor("lvT", [L, SHARD], f32, kind="ExternalOutput")

    exch_in = nc.dram_tensor("exch_in", [PAD_SHARD, F], bf16)
    table2 = nc.dram_tensor("table2", [N_NODES, F], bf16, addr_space="Shared")

    with tile.TileContext(nc) as tc:
        with (
            tc.tile_pool(name="const", bufs=1) as cpool,
            tc.tile_pool(name="sbuf", bufs=2) as sb,
            tc.tile_pool(name="onehot", bufs=6) as ohp,
            tc.tile_pool(name="psum_pre", bufs=2, space="PSUM") as ppre,
            tc.tile_pool(name="psum_blk", bufs=3, space="PSUM") as pblk,
        ):
            iota_row = cpool.tile([P, P], f32)
            nc.gpsimd.iota(iota_row[:], pattern=[[1, P]], base=0,
                           channel_multiplier=0,
                           allow_small_or_imprecise_dtypes=True)
            iota_col = cpool.tile([P, 1], f32)
            nc.gpsimd.iota(iota_col[:], pattern=[[1, 1]], base=0,
                           channel_multiplier=1,
                           allow_small_or_imprecise_dtypes=True)
            ident = cpool.tile([P, P], f32)
            make_identity(nc, ident[:])
            gw1 = cpool.tile([F, F], f32)
            nc.sync.dma_start(out=gw1[:], in_=gw1_d[:, :])
            gw2 = cpool.tile([F, F], f32)
            nc.sync.dma_start(out=gw2[:], in_=gw2_d[:, :])
            gb1 = cpool.tile([F, 1], f32)
            nc.sync.dma_start(out=gb1[:], in_=gb1_d[:, :])
            gb2 = cpool.tile([F, 1], f32)
            nc.sync.dma_start(out=gb2[:], in_=gb2_d[:, :])
            muw = cpool.tile([F, L], f32)
            nc.sync.dma_start(out=muw[:], in_=muw_d[:, :])
            lvw = cpool.tile([F, L], f32)
            nc.sync.dma_start(out=lvw[:], in_=lvw_d[:, :])
            mub = cpool.tile([L, 1], f32)
            nc.sync.dma_start(out=mub[:], in_=mub_d[:, :])
            lvb = cpool.tile([L, 1], f32)
            nc.sync.dma_start(out=lvb[:], in_=lvb_d[:, :])
            dis2_all = cpool.tile([P, NBLK], f32)
            # dis2own is [PAD_SHARD,1] = [NBLK*P,1] -> [P, NBLK] with block b in col b
            nc.sync.dma_start(
                out=dis2_all[:],
                in_=dis2_d.ap().rearrange("(b p) o -> p (b o)", p=P))

            # zero the padded tail of exch_in (L2 self-row reads touch it)
            zt = cpool.tile([P, F], bf16)
            nc.vector.memset(zt[:], 0.0)
            nc.sync.dma_start(out=exch_in[SHARD:PAD_SHARD, :],
                              in_=zt[: PAD_SHARD - SHARD, :])

            layers = (1,) if phase in ("l1",) else (1, 2)
            for layer in layers:
                dt = f32 if layer == 1 else bf16
                gw, gb = (gw1, gb1) if layer == 1 else (gw2, gb2)
                offA = offB = offC = 0
                dbg_nr = int(os.environ.get("KERNEL_DEBUG_RANGES", "0"))
                for r, blks in enumerate(rng_blocks):
                    if dbg_nr and r >= dbg_nr:
                        break
                    nchA = nchA_rng[r]
                    nchB = nchB_rng[r]
                    nA, nB = nchA * P, nchB * P
                    SAr, SBr = nA // 16, nB // 16

                    idxA_t = sb.tile([P, SAr], mybir.dt.int16, tag="idxA")
                    nc.sync.dma_start(out=idxA_t[:], in_=idxA_d[:, offA: offA + SAr])
                    idxB_t = sb.tile([P, SBr], mybir.dt.int16, tag="idxB")
                    nc.sync.dma_start(out=idxB_t[:], in_=idxB_d[:, offB: offB + SBr])
                    ld_t = sb.tile([P, nchA + nchB], f32, tag="ld")
                    nc.sync.dma_start(out=ld_t[:], in_=ld_d[:, offC: offC + nchA + nchB])
                    nm_t = sb.tile([P, nchA + nchB], f32, tag="nm")
                    nc.sync.dma_start(out=nm_t[:], in_=nm_d[:, offC: offC + nchA + nchB])

                    if layer == 1:
                        tabA = table1[:, :]
                        tabB = table1[SPLIT:, :]
                        selfsrc = selfrows_d
                    else:
                        tabA = table2[:, :]
                        tabB = table2[SPLIT:, :]
                        selfsrc = exch_in

                    gbufA = sb.tile([P, nchA, F], dt, tag="gbufA")
                    nc.gpsimd.dma_gather(gbufA[:], tabA, idxA_t[:], nA, nA, F, single_packet=False)
                    gbufB = sb.tile([P, nchB, F], dt, tag="gbufB")
                    nc.gpsimd.dma_gather(gbufB[:], tabB, idxB_t[:], nB, nB, F, single_packet=False)

                    pre = ppre.tile([P, len(blks) * P], f32, space="PSUM",
                                    tag="pre")
                    cA = 0
                    cB = nchA
                    for bi, b in enumerate(blks):
                        ncha = int(nch_s[b, 0])
                        nchb = int(nch_s[b, 1])
                        pre_b = pre[:, bi * P: (bi + 1) * P]
                        nchunks = ncha + nchb + 1
                        ci = 0
                        for k in range(ncha):
                            s_t = ohp.tile([P, P], dt, tag="oh")
                            nc.vector.tensor_scalar(
                                out=s_t[:], in0=iota_row[:],
                                scalar1=ld_t[:, cA + k: cA + k + 1],
                                scalar2=nm_t[:, cA + k: cA + k + 1],
                                op0=mybir.AluOpType.is_equal,
                                op1=mybir.AluOpType.mult)
                            nc.tensor.matmul(out=pre_b, lhsT=gbufA[:, cA + k, :],
                                             rhs=s_t[:], start=(ci == 0),
                                             stop=(ci == nchunks - 1))
                            ci += 1
                        for k in range(nchb):
                            s_t = ohp.tile([P, P], dt, tag="oh")
                            nc.vector.tensor_scalar(
                                out=s_t[:], in0=iota_row[:],
                                scalar1=ld_t[:, cB + k: cB + k + 1],
                                scalar2=nm_t[:, cB + k: cB + k + 1],
                                op0=mybir.AluOpType.is_equal,
                                op1=mybir.AluOpType.mult)
                            nc.tensor.matmul(out=pre_b, lhsT=gbufB[:, cB - nchA + k, :],
                                             rhs=s_t[:], start=(ci == 0),
                                             stop=(ci == nchunks - 1))
                            ci += 1
                        # self-loop diagonal chunk
                        gself = sb.tile([P, F], dt, tag="gself")
                        nc.sync.dma_start(out=gself[:],
                                          in_=selfsrc[b * P: (b + 1) * P, :])
                        s_t = ohp.tile([P, P], dt, tag="oh")
                        nc.vector.tensor_scalar(
                            out=s_t[:], in0=iota_row[:],
                            scalar1=iota_col[:],
                            scalar2=dis2_all[:, b: b + 1],
                            op0=mybir.AluOpType.is_equal,
                            op1=mybir.AluOpType.mult)
                        nc.tensor.matmul(out=pre_b, lhsT=gself[:], rhs=s_t[:],
                                         start=(ci == 0), stop=True)
                        cA += ncha
                        cB += nchb

                        nrows = P if b < NBLK - 1 else LAST_BLK_ROWS
                        # aggpre^T [feat, dst] -> apply W: outT = (aggpre@W)^T
                        preT_sb = sb.tile([P, P], f32, tag="preT")
                        nc.scalar.copy(out=preT_sb[:], in_=pre_b)
                        outT = pblk.tile([P, P], f32, space="PSUM", tag="pblk")
                        nc.tensor.matmul(out=outT[:], lhsT=gw[:], rhs=preT_sb[:],
                                         start=True, stop=True)
                        hT = sb.tile([P, P], f32, tag="hT")
                        nc.scalar.activation(out=hT[:], in_=outT[:],
                                             func=mybir.ActivationFunctionType.Relu,
                                             bias=gb[:], scale=1.0)
                        if layer == 1:
                            # transpose to node-major, cast bf16, store shard
                            tps = pblk.tile([P, P], f32, space="PSUM", tag="pblk")
                            nc.tensor.transpose(out=tps[:], in_=hT[:], identity=ident[:])
                            hbf = sb.tile([P, F], bf16, tag="hbf")
                            nc.vector.tensor_copy(out=hbf[:], in_=tps[:])
                            nc.sync.dma_start(
                                out=exch_in[b * P: b * P + nrows, :],
                                in_=hbf[:nrows, :])
                        else:
                            muT_ps = pblk.tile([L, P], f32, space="PSUM", tag="pblk")
                            nc.tensor.matmul(out=muT_ps[:], lhsT=muw[:], rhs=hT[:],
                                             start=True, stop=True)
                            muT_sb = sb.tile([L, P], f32, tag="muT")
                            nc.scalar.activation(
                                out=muT_sb[:], in_=muT_ps[:],
                                func=mybir.ActivationFunctionType.Identity,
                                bias=mub[:], scale=1.0)
                            nc.sync.dma_start(
                                out=muT_o[:, b * P: b * P + nrows],
                                in_=muT_sb[:, :nrows])
                            lvT_ps = pblk.tile([L, P], f32, space="PSUM", tag="pblk")
                            nc.tensor.matmul(out=lvT_ps[:], lhsT=lvw[:], rhs=hT[:],
                                             start=True, stop=True)
                            lvT_sb = sb.tile([L, P], f32, tag="lvT")
                            nc.scalar.activation(
                                out=lvT_sb[:], in_=lvT_ps[:],
                                func=mybir.ActivationFunctionType.Identity,
                                bias=lvb[:], scale=1.0)
                            nc.sync.dma_start(
                                out=lvT_o[:, b * P: b * P + nrows],
                                in_=lvT_sb[:, :nrows])

                    offA += SAr
                    offB += SBr
                    offC += nchA + nchB

                if layer == 1 and phase not in ("l1", "nocc"):
                    nc.gpsimd.collective_compute(
                        "AllGather",
                        mybir.AluOpType.bypass,
                        replica_groups=[list(range(CORES))],
                        ins=[exch_in[:SHARD, :]],
                        outs=[table2.ap().opt()],
                    )

    nc.compile()
    return nc


def kernel(x, edge_index, homophily_cond, hw1, hb1, hw2, hb2,
           gw1, gb1, gw2, gb2, muw, mub, lvw, lvb):
    H0, per_core, meta = _host_prepare(
        np.asarray(x), np.asarray(edge_index), np.asarray(homophily_cond),
        np.asarray(hw1), np.asarray(hb1), np.asarray(hw2), np.asarray(hb2))

    phase = os.environ.get("KERNEL_DEBUG_PHASE", "full")
    key = ("prog", phase, os.environ.get("KERNEL_DEBUG_RANGES", "0"),
           tuple(meta["nch_s"].reshape(-1).tolist()))
    if key not in _cache:
        _cache[key] = _build_program(meta, phase)
    nc = _cache[key]

    common = dict(
        table1=H0,
        gw1=np.asarray(gw1, np.float32), gw2=np.asarray(gw2, np.float32),
        gb1=np.asarray(gb1, np.float32).reshape(F, 1),
        gb2=np.asarray(gb2, np.float32).reshape(F, 1),
        muw=np.asarray(muw, np.float32), lvw=np.asarray(lvw, np.float32),
        mub=np.asarray(mub, np.float32).reshape(L, 1),
        lvb=np.asarray(lvb, np.float32).reshape(L, 1),
    )
    in_maps = []
    for c in range(CORES):
        m = dict(common)
        pc = per_core[c]
        m.update(idxA=pc["idxA"], idxB=pc["idxB"], ldm=pc["ldm"], nmm=pc["nmm"],
                 selfrows=pc["selfrows"], dis2own=pc["dis2own"])
        in_maps.append(m)

    trace = bool(os.environ.get("BASS_TRACE"))
    if trace:
        _install_trace_shims()
    ncores = int(os.environ.get("KERNEL_DEBUG_CORES", str(CORES)))
    tmpdir = os.environ.get("KERNEL_TRACE_DIR") or None
    res = run_bass_kernel_spmd(nc, in_maps[:ncores], core_ids=list(range(ncores)),
                               trace=trace, tmpdir=tmpdir)
    last_run_info["exec_time_ns"] = res.exec_time_ns
    last_run_info["results"] = res

    nres = len(res.results)
    mu = np.concatenate([res.results[c % nres]["muT"].T for c in range(CORES)], axis=0)
    lv = np.concatenate([res.results[c % nres]["lvT"].T for c in range(CORES)], axis=0)
    return mu.astype(np.float32), lv.astype(np.float32)


# revision 7
# speedup vs baseline: 2.0205x; 2.0205x over previous
"""Trainium2 Bass kernel for ConditionalStructureEncoder (2-layer GCN + VAE heads).

Strategy (8 NeuronCores, SPMD):
  - Destination-node sharding: core c owns dst nodes [c*6250, (c+1)*6250).
  - Host precomputes: homophily MLP folded into node features (H0 = x + hom),
    GCN symmetric norm, per-core edge lists sorted by (dst block, src-half)
    padded to 128-edge chunks with uniform static chunk counts across cores
    (SPMD requires an identical program), and the per-chunk selection
    matrices S[e,d] = norm_e * onehot(localdst_e) in bf16 (shared by both
    GCN layers; streamed in by DMA so no on-device one-hot building).
  - Aggregation A_hat @ H as PE matmuls: per 128-edge chunk,
    psum[feat, dst-block] += G_chunk[e,:].T @ S_chunk[e,:], where G rows
    come from batched SWDGE dma_gather (bf16 tables, 4 parallel queues).
    Self-loops are one extra diagonal chunk per block (S = diag(dis^2)).
  - The layer weight W is applied AFTER aggregation (linearity), so the
    gather table holds raw activations and per-block matmuls are tiny.
  - Inter-layer halo exchange: AllGather of layer-1 activations in bf16.
  - mu/logvar heads per block in transposed layout; host de-transposes.

Self-contained: hardcodes all shapes from the problem spec.
"""
import os
import sys

sys.path.insert(0, "/opt/trn_rl_repo")

import numpy as np
import ml_dtypes

import concourse.bass as bass
import concourse.mybir as mybir
from concourse import bacc, tile
from concourse.bass_utils import run_bass_kernel_spmd

P = 128
N_NODES = 50000
F = 128
L = 64
CORES = 8
SHARD = N_NODES // CORES            # 6250
NBLK = (SHARD + P - 1) // P         # 49 blocks (48 full + 1 of 106)
LAST_BLK_ROWS = SHARD - (NBLK - 1) * P  # 106
RPB = 4                             # dst blocks per gather range
NRANGES = (NBLK + RPB - 1) // RPB   # 13
SPLIT = 32768                       # int16 gather index limit
PAD_SHARD = NBLK * P                # 6272
NQ = int(os.environ.get("KERNEL_NQ", "4"))   # SWDGE queues

_cache = {}

# info for test harness
last_run_info = {}


def _install_trace_shims():
    """Register the NTFF profile hook (missing antenv.axon_hooks in this
    container) and neuter the artifact upload so trace=True works."""
    import types
    import contextlib
    import ctypes

    import antenv
    from concourse import bass_utils as bu

    if "antenv.axon_hooks" not in sys.modules:
        mod = types.ModuleType("antenv.axon_hooks")
        _state = {"hook": None}

        def set_axon_ntff_profile_hook(h):
            _state["hook"] = h

        def get_axon_ntff_profile_hook():
            return _state["hook"]

        mod.set_axon_ntff_profile_hook = set_axon_ntff_profile_hook
        mod.get_axon_ntff_profile_hook = get_axon_ntff_profile_hook
        sys.modules["antenv.axon_hooks"] = mod
        antenv.axon_hooks = mod

        lib = ctypes.CDLL("/opt/axon/libaxon_pjrt.so")
        lib.axon_start_nrt_profile.argtypes = [
            ctypes.POINTER(ctypes.c_int64), ctypes.c_size_t]
        lib.axon_start_nrt_profile.restype = ctypes.c_int64
        lib.axon_stop_nrt_profile.argtypes = [ctypes.c_char_p]
        lib.axon_stop_nrt_profile.restype = ctypes.c_int64

        @contextlib.contextmanager
        def _hook(output_dir, device_ids):
            import jax
            jax.devices()
            if device_ids:
                ids = (ctypes.c_int64 * len(device_ids))(*device_ids)
                rc = lib.axon_start_nrt_profile(ids, len(device_ids))
            else:
                rc = lib.axon_start_nrt_profile(None, 0)
            if rc != 0:
                raise RuntimeError(f"axon_start_nrt_profile rc={rc}")
            try:
                yield
            finally:
                n = lib.axon_stop_nrt_profile(str(output_dir).encode())
                print(f"profile: {n} file(s) written to {output_dir}")

        set_axon_ntff_profile_hook(_hook)

    bu.upload_artifacts = lambda tmpdir: tmpdir


def _host_prepare(x, edge_index, homophily_cond, hw1, hb1, hw2, hb2):
    """Compute H0 = x + hom and all per-core edge structures."""
    hc = homophily_cond.astype(np.float64)[None, :]
    hom = np.maximum(hc @ hw1.astype(np.float64) + hb1, 0.0) @ hw2.astype(np.float64) + hb2
    H0 = (x.astype(np.float64) + hom).astype(np.float32)  # [N, F]

    src = edge_index[0].astype(np.int64)
    dst = edge_index[1].astype(np.int64)

    deg = np.bincount(dst, minlength=N_NODES).astype(np.float64) + 1.0
    dis = 1.0 / np.sqrt(deg)
    norm_e = (dis[src] * dis[dst]).astype(np.float32)
    dis2 = (dis * dis).astype(np.float32)

    core = dst // SHARD
    localdst = dst % SHARD
    blk = localdst // P
    reldst = (localdst % P).astype(np.int64)
    isB = (src >= SPLIT).astype(np.int64)
    group = (core * NBLK + blk) * 2 + isB
    order = np.argsort(group, kind="stable")
    src_sorted = src[order]
    rel_sorted = reldst[order]
    nrm_sorted = norm_e[order]
    counts = np.bincount(group, minlength=CORES * NBLK * 2)
    starts = np.zeros(CORES * NBLK * 2 + 1, np.int64)
    np.cumsum(counts, out=starts[1:])

    cnt = counts.reshape(CORES, NBLK, 2)
    nch = (cnt + P - 1) // P
    nch_s = nch.max(axis=0)                        # static chunk counts [NBLK, 2]

    rng_blocks = [list(range(r * RPB, min((r + 1) * RPB, NBLK)))
                  for r in range(NRANGES)]
    nchA_rng = [int(sum(nch_s[b, 0] for b in blks)) for blks in rng_blocks]
    nchB_rng = [int(sum(nch_s[b, 1] for b in blks)) for blks in rng_blocks]

    per_core = []
    for c in range(CORES):
        idxA_parts, idxB_parts = [], []
        s_parts = []
        for r, blks in enumerate(rng_blocks):
            iA, iB = [], []
            sA, sB = [], []
            for b in blks:
                for half, (ilist, slist) in enumerate(((iA, sA), (iB, sB))):
                    g = (c * NBLK + b) * 2 + half
                    s0, e0 = starts[g], starts[g + 1]
                    n_real = int(e0 - s0)
                    n_pad = int(nch_s[b, half]) * P
                    sidx = np.zeros(n_pad, np.int64)
                    sidx[:n_real] = src_sorted[s0:e0] - (SPLIT if half else 0)
                    ilist.append(sidx.astype(np.int16))
                    # S block: for each chunk a [128 e, 128 d] matrix
                    S = np.zeros((n_pad, P), np.float32)
                    if n_real:
                        S[np.arange(n_real), rel_sorted[s0:e0]] = nrm_sorted[s0:e0]
                    # edge j -> partition j%128, chunk j//128
                    s_blk = S.reshape(-1, P, P).transpose(1, 0, 2).reshape(P, -1)
                    slist.append(s_blk)
            # self-loop diagonal chunks, one per block in this range
            selfS = []
            for b in blks:
                d2 = np.zeros(P, np.float32)
                lo = c * SHARD + b * P
                hi = min(c * SHARD + (b + 1) * P, (c + 1) * SHARD)
                d2[: hi - lo] = dis2[lo:hi]
                selfS.append(np.diag(d2).astype(np.float32))
            s_parts.append(np.concatenate(sA + sB + selfS, axis=1))

            def wrap_idx(cols):
                flat = np.concatenate(cols)
                m = flat.reshape(-1, 16).T
                return np.tile(m, (8, 1))

            idxA_parts.append(wrap_idx(iA))
            idxB_parts.append(wrap_idx(iB))

        idxA = np.concatenate(idxA_parts, axis=1)
        idxB = np.concatenate(idxB_parts, axis=1)
        smat = np.concatenate(s_parts, axis=1).astype(ml_dtypes.bfloat16)

        selfrows = np.zeros((PAD_SHARD, F), np.float32)
        selfrows[:SHARD] = H0[c * SHARD: (c + 1) * SHARD]
        per_core.append(dict(idxA=idxA, idxB=idxB, smat=smat,
                             selfrows=selfrows.astype(ml_dtypes.bfloat16)))

    meta = dict(nch_s=nch_s, rng_blocks=rng_blocks,
                nchA_rng=nchA_rng, nchB_rng=nchB_rng)
    return H0, per_core, meta


def _qsplit(nch):
    """Split nch chunks into NQ contiguous groups (chunk counts)."""
    base = nch // NQ
    rem = nch % NQ
    return [base + (1 if i < rem else 0) for i in range(NQ)]


def _build_program(meta, phase="full"):
    nch_s = meta["nch_s"]
    rng_blocks = meta["rng_blocks"]
    nchA_rng = meta["nchA_rng"]
    nchB_rng = meta["nchB_rng"]
    SA_tot = sum(n * P // 16 for n in nchA_rng)
    SB_tot = sum(n * P // 16 for n in nchB_rng)
    SCOL_tot = sum((nchA_rng[r] + nchB_rng[r] + len(rng_blocks[r])) * P
                   for r in range(NRANGES))
    f32 = mybir.dt.float32
    bf16 = mybir.dt.bfloat16

    nc = bacc.Bacc("TRN2", target_bir_lowering=False, debug=False,
                   num_devices=CORES, num_swdge_queues=NQ)

    table1 = nc.dram_tensor("table1", [N_NODES, F], bf16, kind="ExternalInput")
    idxA_d = nc.dram_tensor("idxA", [P, SA_tot], mybir.dt.int16, kind="ExternalInput")
    idxB_d = nc.dram_tensor("idxB", [P, SB_tot], mybir.dt.int16, kind="ExternalInput")
    smat_d = nc.dram_tensor("smat", [P, SCOL_tot], bf16, kind="ExternalInput")
    selfrows_d = nc.dram_tensor("selfrows", [PAD_SHARD, F], bf16, kind="ExternalInput")
    gw1_d = nc.dram_tensor("gw1", [F, F], f32, kind="ExternalInput")
    gw2_d = nc.dram_tensor("gw2", [F, F], f32, kind="ExternalInput")
    gb1_d = nc.dram_tensor("gb1", [F, 1], f32, kind="ExternalInput")
    gb2_d = nc.dram_tensor("gb2", [F, 1], f32, kind="ExternalInput")
    muw_d = nc.dram_tensor("muw", [F, L], f32, kind="ExternalInput")
    lvw_d = nc.dram_tensor("lvw", [F, L], f32, kind="ExternalInput")
    mub_d = nc.dram_tensor("mub", [L, 1], f32, kind="ExternalInput")
    lvb_d = nc.dram_tensor("lvb", [L, 1], f32, kind="ExternalInput")
    muT_o = nc.dram_tensor("muT", [L, SHARD], f32, kind="ExternalOutput")
    lvT_o = nc.dram_tensor("lvT", [L, SHARD], f32, kind="ExternalOutput")

    exch_in = nc.dram_tensor("exch_in", [PAD_SHARD, F], bf16)
    table2 = nc.dram_tensor("table2", [N_NODES, F], bf16, addr_space="Shared")

    with tile.TileContext(nc) as tc:
        with (
            tc.tile_pool(name="const", bufs=1) as cpool,
            tc.tile_pool(name="sbuf", bufs=2) as sb,
            tc.tile_pool(name="psum_pre", bufs=2, space="PSUM") as ppre,
            tc.tile_pool(name="psum_blk", bufs=3, space="PSUM") as pblk,
        ):
            ident = cpool.tile([P, P], f32)
            from concourse.masks import make_identity
            make_identity(nc, ident[:])
            gw1 = cpool.tile([F, F], f32)
            nc.sync.dma_start(out=gw1[:], in_=gw1_d[:, :])
            gw2 = cpool.tile([F, F], f32)
            nc.sync.dma_start(out=gw2[:], in_=gw2_d[:, :])
            gb1 = cpool.tile([F, 1], f32)
            nc.sync.dma_start(out=gb1[:], in_=gb1_d[:, :])
            gb2 = cpool.tile([F, 1], f32)
            nc.sync.dma_start(out=gb2[:], in_=gb2_d[:, :])
            muw = cpool.tile([F, L], f32)
            nc.sync.dma_start(out=muw[:], in_=muw_d[:, :])
            lvw = cpool.tile([F, L], f32)
            nc.sync.dma_start(out=lvw[:], in_=lvw_d[:, :])
            mub = cpool.tile([L, 1], f32)
            nc.sync.dma_start(out=mub[:], in_=mub_d[:, :])
            lvb = cpool.tile([L, 1], f32)
            nc.sync.dma_start(out=lvb[:], in_=lvb_d[:, :])

            # zero the padded tail of exch_in (L2 self-row reads touch it)
            zt = cpool.tile([P, F], bf16)
            nc.vector.memset(zt[:], 0.0)
            nc.sync.dma_start(out=exch_in[SHARD:PAD_SHARD, :],
                              in_=zt[: PAD_SHARD - SHARD, :])

            layers = (1,) if phase in ("l1",) else (1, 2)
            for layer in layers:
                gw, gb = (gw1, gb1) if layer == 1 else (gw2, gb2)
                offA = offB = offS = 0
                dbg_nr = int(os.environ.get("KERNEL_DEBUG_RANGES", "0"))
                for r, blks in enumerate(rng_blocks):
                    if dbg_nr and r >= dbg_nr:
                        break
                    nchA = nchA_rng[r]
                    nchB = nchB_rng[r]
                    SAr, SBr = nchA * 8, nchB * 8
                    scols = (nchA + nchB + len(blks)) * P

                    idxA_t = sb.tile([P, SAr], mybir.dt.int16, tag="idxA")
                    nc.sync.dma_start(out=idxA_t[:], in_=idxA_d[:, offA: offA + SAr])
                    idxB_t = sb.tile([P, SBr], mybir.dt.int16, tag="idxB")
                    nc.sync.dma_start(out=idxB_t[:], in_=idxB_d[:, offB: offB + SBr])
                    s_t = sb.tile([P, scols], bf16, tag="smat")
                    nc.sync.dma_start(out=s_t[:], in_=smat_d[:, offS: offS + scols])

                    if layer == 1:
                        tabA = table1[:, :]
                        tabB = table1[SPLIT:, :]
                        selfsrc = selfrows_d
                    else:
                        tabA = table2[:, :]
                        tabB = table2[SPLIT:, :]
                        selfsrc = exch_in

                    gbufA = sb.tile([P, nchA, F], bf16, tag="gbufA")
                    c0 = 0
                    for q, qn in enumerate(_qsplit(nchA)):
                        if qn == 0:
                            continue
                        nc.gpsimd.dma_gather(
                            gbufA[:, c0: c0 + qn, :], tabA,
                            idxA_t[:, c0 * 8: (c0 + qn) * 8],
                            qn * P, qn * P, F,
                            single_packet=False, queue_num=q)
                        c0 += qn
                    gbufB = sb.tile([P, nchB, F], bf16, tag="gbufB")
                    c0 = 0
                    for q, qn in enumerate(_qsplit(nchB)):
                        if qn == 0:
                            continue
                        nc.gpsimd.dma_gather(
                            gbufB[:, c0: c0 + qn, :], tabB,
                            idxB_t[:, c0 * 8: (c0 + qn) * 8],
                            qn * P, qn * P, F,
                            single_packet=False, queue_num=q)
                        c0 += qn

                    pre = ppre.tile([P, len(blks) * P], f32, space="PSUM",
                                    tag="pre")
                    cA = 0
                    cB = nchA
                    for bi, b in enumerate(blks):
                        ncha = int(nch_s[b, 0])
                        nchb = int(nch_s[b, 1])
                        pre_b = pre[:, bi * P: (bi + 1) * P]
                        nchunks = ncha + nchb + 1
                        ci = 0
                        for k in range(ncha):
                            nc.tensor.matmul(
                                out=pre_b, lhsT=gbufA[:, cA + k, :],
                                rhs=s_t[:, (cA + k) * P: (cA + k + 1) * P],
                                start=(ci == 0), stop=(ci == nchunks - 1))
                            ci += 1
                        for k in range(nchb):
                            nc.tensor.matmul(
                                out=pre_b, lhsT=gbufB[:, cB - nchA + k, :],
                                rhs=s_t[:, (cB + k) * P: (cB + k + 1) * P],
                                start=(ci == 0), stop=(ci == nchunks - 1))
                            ci += 1
                        # self-loop diagonal chunk
                        gself = sb.tile([P, F], bf16, tag="gself")
                        nc.sync.dma_start(out=gself[:],
                                          in_=selfsrc[b * P: (b + 1) * P, :])
                        scol0 = (nchA + nchB + bi) * P
                        nc.tensor.matmul(
                            out=pre_b, lhsT=gself[:],
                            rhs=s_t[:, scol0: scol0 + P],
                            start=(ci == 0), stop=True)
                        cA += ncha
                        cB += nchb

                        nrows = P if b < NBLK - 1 else LAST_BLK_ROWS
                        preT_sb = sb.tile([P, P], f32, tag="preT")
                        nc.scalar.copy(out=preT_sb[:], in_=pre_b)
                        outT = pblk.tile([P, P], f32, space="PSUM", tag="pblk")
                        nc.tensor.matmul(out=outT[:], lhsT=gw[:], rhs=preT_sb[:],
                                         start=True, stop=True)
                        hT = sb.tile([P, P], f32, tag="hT")
                        nc.scalar.activation(out=hT[:], in_=outT[:],
                                             func=mybir.ActivationFunctionType.Relu,
                                             bias=gb[:], scale=1.0)
                        if layer == 1:
                            tps = pblk.tile([P, P], f32, space="PSUM", tag="pblk")
                            nc.tensor.transpose(out=tps[:], in_=hT[:],
                                                identity=ident[:])
                            hbf = sb.tile([P, F], bf16, tag="hbf")
                            nc.vector.tensor_copy(out=hbf[:], in_=tps[:])
                            nc.sync.dma_start(
                                out=exch_in[b * P: b * P + nrows, :],
                                in_=hbf[:nrows, :])
                        else:
                            muT_ps = pblk.tile([L, P], f32, space="PSUM", tag="pblk")
                            nc.tensor.matmul(out=muT_ps[:], lhsT=muw[:], rhs=hT[:],
                                             start=True, stop=True)
                            muT_sb = sb.tile([L, P], f32, tag="muT")
                            nc.scalar.activation(
                                out=muT_sb[:], in_=muT_ps[:],
                                func=mybir.ActivationFunctionType.Identity,
                                bias=mub[:], scale=1.0)
                            nc.sync.dma_start(
                                out=muT_o[:, b * P: b * P + nrows],
                                in_=muT_sb[:, :nrows])
                            lvT_ps = pblk.tile([L, P], f32, space="PSUM", tag="pblk")
                            nc.tensor.matmul(out=lvT_ps[:], lhsT=lvw[:], rhs=hT[:],
                                             start=True, stop=True)
                            lvT_sb = sb.tile([L, P], f32, tag="lvT")
                            nc.scalar.activation(
                                out=lvT_sb[:], in_=lvT_ps[:],
                                func=mybir.ActivationFunctionType.Identity,
                                bias=lvb[:], scale=1.0)
                            nc.sync.dma_start(
                                out=lvT_o[:, b * P: b * P + nrows],
                                in_=lvT_sb[:, :nrows])

                    offA += SAr
                    offB += SBr
                    offS += scols

                if layer == 1 and phase not in ("l1", "nocc"):
                    nc.gpsimd.collective_compute(
                        "AllGather",
                        mybir.AluOpType.bypass,
                        replica_groups=[list(range(CORES))],
                        ins=[exch_in[:SHARD, :]],
                        outs=[table2.ap().opt()],
                    )

    nc.compile()
    return nc


def kernel(x, edge_index, homophily_cond, hw1, hb1, hw2, hb2,
           gw1, gb1, gw2, gb2, muw, mub, lvw, lvb):
    H0, per_core, meta = _host_prepare(
        np.asarray(x), np.asarray(edge_index), np.asarray(homophily_cond),
        np.asarray(hw1), np.asarray(hb1), np.asarray(hw2), np.asarray(hb2))

    phase = os.environ.get("KERNEL_DEBUG_PHASE", "full")
    key = ("prog", phase, NQ, os.environ.get("KERNEL_DEBUG_RANGES", "0"),
           tuple(meta["nch_s"].reshape(-1).tolist()))
    if key not in _cache:
        _cache[key] = _build_program(meta, phase)
    nc = _cache[key]

    common = dict(
        table1=H0.astype(ml_dtypes.bfloat16),
        gw1=np.asarray(gw1, np.float32), gw2=np.asarray(gw2, np.float32),
        gb1=np.asarray(gb1, np.float32).reshape(F, 1),
        gb2=np.asarray(gb2, np.float32).reshape(F, 1),
        muw=np.asarray(muw, np.float32), lvw=np.asarray(lvw, np.float32),
        mub=np.asarray(mub, np.float32).reshape(L, 1),
        lvb=np.asarray(lvb, np.float32).reshape(L, 1),
    )
    in_maps = []
    for c in range(CORES):
        m = dict(common)
        pc = per_core[c]
        m.update(idxA=pc["idxA"], idxB=pc["idxB"], smat=pc["smat"],
                 selfrows=pc["selfrows"])
        in_maps.append(m)

    trace = bool(os.environ.get("BASS_TRACE"))
    if trace:
        _install_trace_shims()
    ncores = int(os.environ.get("KERNEL_DEBUG_CORES", str(CORES)))
    tmpdir = os.environ.get("KERNEL_TRACE_DIR") or None
    res = run_bass_kernel_spmd(nc, in_maps[:ncores], core_ids=list(range(ncores)),
                               trace=trace, tmpdir=tmpdir)
    last_run_info["exec_time_ns"] = res.exec_time_ns
    last_run_info["results"] = res

    nres = len(res.results)
    mu = np.concatenate([res.results[c % nres]["muT"].T for c in range(CORES)], axis=0)
    lv = np.concatenate([res.results[c % nres]["lvT"].T for c in range(CORES)], axis=0)
    return mu.astype(np.float32), lv.astype(np.float32)
